# revision 19
# baseline (speedup 1.0000x reference)
"""Trainium2 Bass kernel for nn_CNN_88098369175780.

Strategy (8 NeuronCores, two NEFF launches, no collectives):
  Launch 1 (SPMD x8): sequence-parallel attention. Each core owns a 514-wide
  q-slice (512 + 2 halo columns so the conv stack needs no cross-core halo).
  The T x T matrices are never materialized in HBM; scores are computed in
  transposed orientation (keys on partitions) in bf16, softmax uses the
  algebraic upper bound 6*sum(Q_row) as the shift (K <= 6, Q >= 0 makes it a
  true bound; no row-max reduction needed), the row-sum rides along as an
  extra ones-column of V, and wavP @ (eeg2.T @ wavP) is reassociated through
  the 16x16 Gram matrix. Each core then runs conv0-conv2 on its local slice
  (perfectly aligned: 512 conv0 cols -> 128 conv1 -> 32 conv2).
  Launch 2 (1 core): conv3 + FC head on the host-concatenated [30, 255]
  conv2 map -> [42, 2] output.
"""
import contextlib
import ctypes
import os
import sys
import types

import numpy as np

for _p in ('/root/.axon_site', '/root/.axon_site/_ro/trn_rl_repo',
           '/root/.axon_site/_ro/pypackages', '/opt/trn_rl_repo'):
    if os.path.isdir(_p) and _p not in sys.path:
        sys.path.append(_p)

import ml_dtypes
import concourse.bacc as bacc
import concourse.tile as tile
import concourse.mybir as mybir
from concourse.bass_utils import run_bass_kernel_spmd

f32 = mybir.dt.float32
bf16 = mybir.dt.bfloat16
AF = mybir.ActivationFunctionType
ALU = mybir.AluOpType
BF = ml_dtypes.bfloat16

T = 4096
NC = 8
QN = 514


# ---------------------------------------------------------------- NTFF shim
def _install_ntff_shim():
    name = "antenv.axon_hooks"
    if name in sys.modules:
        return
    so_path = "/opt/axon/libaxon_pjrt.so"
    hook = None
    if os.path.exists(so_path):
        lib = ctypes.CDLL(so_path)
        if hasattr(lib, "axon_start_nrt_profile"):
            lib.axon_start_nrt_profile.argtypes = [
                ctypes.POINTER(ctypes.c_int64), ctypes.c_size_t]
            lib.axon_start_nrt_profile.restype = ctypes.c_int64
            lib.axon_stop_nrt_profile.argtypes = [ctypes.c_char_p]
            lib.axon_stop_nrt_profile.restype = ctypes.c_int64

            @contextlib.contextmanager
            def _hook(output_dir, device_ids):
                import jax
                jax.devices()
                if device_ids:
                    ids = (ctypes.c_int64 * len(device_ids))(*device_ids)
                    rc = lib.axon_start_nrt_profile(ids, len(device_ids))
                else:
                    rc = lib.axon_start_nrt_profile(None, 0)
                if rc != 0:
                    raise RuntimeError(f"axon_start_nrt_profile rc={rc}")
                try:
                    yield
                finally:
                    n = lib.axon_stop_nrt_profile(str(output_dir).encode())
                    if n < 0:
                        raise RuntimeError(f"axon_stop_nrt_profile rc={n}")
            hook = _hook
    mod = types.ModuleType(name)
    mod._hook = hook
    mod.set_axon_ntff_profile_hook = lambda h: setattr(mod, "_hook", h)
    mod.get_axon_ntff_profile_hook = lambda: mod._hook
    sys.modules[name] = mod


_install_ntff_shim()


# ------------------------------------------------------------- host consts
def build_consts(x, cm1_W, cm1_b, cm2_W, cm2_b, cw0, cw1, cw2, cw3, cb,
                 fc1_W, fc1_b, fc2_W, fc2_b):
    F = np.float32
    x = np.asarray(x, F)
    eeg2 = np.ascontiguousarray(x[0, 0, 1:-1, :]).astype(F)
    wavA = np.ascontiguousarray(x[0, 0, 0, :]).astype(F)
    wavB = np.ascontiguousarray(x[0, 0, -1, :]).astype(F)
    cm1_W = np.asarray(cm1_W, F); cm1_b = np.asarray(cm1_b, F)
    cm2_W = np.asarray(cm2_W, F); cm2_b = np.asarray(cm2_b, F)
    cw0 = np.asarray(cw0, F); cw1 = np.asarray(cw1, F)
    cw2 = np.asarray(cw2, F); cw3 = np.asarray(cw3, F); cb = np.asarray(cb, F)
    fc1_W = np.asarray(fc1_W, F); fc1_b = np.asarray(fc1_b, F)
    fc2_W = np.asarray(fc2_W, F); fc2_b = np.asarray(fc2_b, F)

    c = {}
    c['E_aug'] = np.concatenate([eeg2, np.ones((1, T), F)], 0)
    E_slices = []
    for ci in range(NC):
        sl = np.zeros((17, QN), F)
        n = min(QN, T - 512 * ci)
        sl[:, :n] = c['E_aug'][:, 512 * ci:512 * ci + n]
        E_slices.append(sl)
    c['E_slice'] = E_slices
    et = np.transpose(eeg2.reshape(16, 32, 128), (2, 1, 0))
    c['ET_dup'] = np.concatenate([et, et], axis=2).reshape(128, 1024).astype(F)
    wa = wavA.reshape(32, 128).T[:, :, None]
    wb = wavB.reshape(32, 128).T[:, :, None]
    c['wav_exp'] = np.concatenate(
        [np.repeat(wa, 16, 2), np.repeat(wb, 16, 2)], axis=2).reshape(128, 1024).astype(F)
    wb49 = np.zeros((49, T), F)
    wb49[0:16] = wavA[None, :]; wb49[32:48] = wavB[None, :]
    wb49[16] = 1.0; wb49[48] = 1.0
    c['wav_b49'] = wb49
    lk = np.zeros((49, 48), F)
    lk[0:16, 0:16] = cm1_W[1].T; lk[16, 0:16] = cm1_b[1]
    lk[32:48, 32:48] = cm2_W[1].T; lk[48, 32:48] = cm2_b[1]
    c['lhsK'] = lk
    rv = np.zeros((49, 66), F)
    rv[0:16, 0:16] = cm1_W[2].T; rv[16, 0:16] = cm1_b[2]; rv[16, 32] = 1.0
    rv[32:48, 33:49] = cm2_W[2].T; rv[48, 33:49] = cm2_b[2]; rv[48, 65] = 1.0
    c['rhsV49'] = rv
    lq = np.zeros((17, 48), F)
    lq[0:16, 0:16] = cm1_W[0].T; lq[16, 0:16] = cm1_b[0]
    lq[0:16, 32:48] = cm2_W[0].T; lq[16, 32:48] = cm2_b[0]
    c['lhsQ'] = lq
    c['ones16'] = np.ones((16, 1), BF)
    c['neg6'] = np.full((1, T), -6.0, BF)
    g2i = np.zeros((17, 49), F)
    g2i[16, 16] = 1.0; g2i[16, 48] = 1.0
    c['G2init'] = g2i
    c['W3A'] = np.concatenate([cm1_W[3].T, cm1_b[3][None, :]], 0).astype(BF)
    c['W3B'] = np.concatenate([cm2_W[3].T, cm2_b[3][None, :]], 0).astype(BF)

    def y48row(origH):
        if 16 <= origH < 32:
            return origH - 16
        if origH < 16:
            return origH + 16
        return origH
    c0 = np.zeros((3, 49, 120), F)
    for dw in range(3):
        for cch in range(5):
            for h in range(24):
                m = cch * 24 + h
                for dh in range(2):
                    c0[dw, y48row(2 * h + dh), m] += cw0[cch, 0, dh, dw]
                if dw == 0:
                    c0[dw, 48, m] += cb[0][cch]
    c['c0w'] = c0
    c1 = np.zeros((4, 121, 60), F)
    for dw in range(4):
        for cch in range(5):
            for h in range(12):
                m = cch * 12 + h
                for cin in range(5):
                    for dh in range(2):
                        c1[dw, cin * 24 + 2 * h + dh, m] += cw1[cch, cin, dh, dw]
                if dw == 0:
                    c1[dw, 120, m] += cb[1][cch]
    c['c1w'] = c1
    c2 = np.zeros((4, 61, 30), F)
    for dw in range(4):
        for cch in range(5):
            for h in range(6):
                m = cch * 6 + h
                for cin in range(5):
                    for dh in range(2):
                        c2[dw, cin * 12 + 2 * h + dh, m] += cw2[cch, cin, dh, dw]
                if dw == 0:
                    c2[dw, 60, m] += cb[2][cch]
    c['c2w'] = c2
    c3 = np.zeros((4, 31, 15), F)
    for dw in range(4):
        for cch in range(5):
            for h in range(3):
                m = cch * 3 + h
                for cin in range(5):
                    for dh in range(2):
                        c3[dw, cin * 6 + 2 * h + dh, m] += cw3[cch, cin, dh, dw]
                if dw == 0:
                    c3[dw, 30, m] += cb[3][cch]
    c['c3w'] = c3
    c['f1w'] = np.concatenate([fc1_W.T, fc1_b[None, :]], 0).astype(F)
    w_d = np.stack([fc2_W[0] - fc2_W[1], fc2_W[1] - fc2_W[0]], 1)
    b_d = np.array([fc2_b[0] - fc2_b[1], fc2_b[1] - fc2_b[0]], F)
    c['f2w'] = np.concatenate([w_d, b_d[None, :]], 0).astype(F)
    return c


# ---------------------------------------------------------------- launch 1
def _build_launch1():
    nc = bacc.Bacc("TRN2", target_bir_lowering=False, debug=False,
                   num_devices=NC)
    dt = nc.dram_tensor
    a = {
        'ET_dup':  dt('ET_dup',  [128, 1024], bf16, kind="ExternalInput").ap(),
        'wav_exp': dt('wav_exp', [128, 1024], bf16, kind="ExternalInput").ap(),
        'E_aug':   dt('E_aug',   [17, T],     bf16, kind="ExternalInput").ap(),
        'wav_b49': dt('wav_b49', [49, T],     bf16, kind="ExternalInput").ap(),
        'G2init':  dt('G2init',  [17, 49],    bf16, kind="ExternalInput").ap(),
        'lhsK':    dt('lhsK',    [49, 48],    bf16, kind="ExternalInput").ap(),
        'E_slice': dt('E_slice', [17, QN],    bf16, kind="ExternalInput").ap(),
        'lhsQ':    dt('lhsQ',    [17, 48],    bf16, kind="ExternalInput").ap(),
        'ones16':  dt('ones16',  [16, 1],     bf16, kind="ExternalInput").ap(),
        'neg6':    dt('neg6',    [1, T],      bf16, kind="ExternalInput").ap(),
        'rhsV49':  dt('rhsV49',  [49, 66],    bf16, kind="ExternalInput").ap(),
        'W3A':     dt('W3A',     [17, 16],    bf16, kind="ExternalInput").ap(),
        'W3B':     dt('W3B',     [17, 16],    bf16, kind="ExternalInput").ap(),
        'c0w':     dt('c0w',     [3, 49, 120], bf16, kind="ExternalInput").ap(),
        'c1w':     dt('c1w',     [4, 121, 60], bf16, kind="ExternalInput").ap(),
        'c2w':     dt('c2w',     [4, 61, 30],  bf16, kind="ExternalInput").ap(),
        'oconv2':  dt('oconv2',  [30, 32],    f32, kind="ExternalOutput").ap(),
    }

    with tile.TileContext(nc) as tc:
        with tc.tile_pool(name="const", bufs=1) as cp, \
             tc.tile_pool(name="work", bufs=2) as wp, \
             tc.tile_pool(name="exps", bufs=3) as ep, \
             tc.tile_pool(name="psumP", bufs=2, space="PSUM") as psP, \
             tc.tile_pool(name="psumUA", bufs=1, space="PSUM") as psUA, \
             tc.tile_pool(name="psumUB", bufs=1, space="PSUM") as psUB, \
             tc.tile_pool(name="psumH", bufs=1, space="PSUM") as psH, \
             tc.tile_pool(name="psumS", bufs=1, space="PSUM") as psS:

            # critical loads first (split for early dependency release)
            ET_dup = cp.tile([128, 1024], bf16, tag="ET_dup")
            wav_exp = cp.tile([128, 1024], bf16, tag="wav_exp")
            nc.sync.dma_start(ET_dup[:, 0:512], a['ET_dup'][:, 0:512])
            nc.sync.dma_start(wav_exp[:, 0:512], a['wav_exp'][:, 0:512])
            nc.sync.dma_start(ET_dup[:, 512:1024], a['ET_dup'][:, 512:1024])
            nc.sync.dma_start(wav_exp[:, 512:1024], a['wav_exp'][:, 512:1024])
            E_aug = cp.tile([17, T], bf16, tag="E_aug")
            nc.sync.dma_start(E_aug[:, 0:2048], a['E_aug'][:, 0:2048])
            nc.sync.dma_start(E_aug[:, 2048:T], a['E_aug'][:, 2048:T])
            wav_b49 = cp.tile([49, T], bf16, tag="wav_b49")
            nc.sync.dma_start(wav_b49[:, 0:2048], a['wav_b49'][:, 0:2048])
            nc.sync.dma_start(wav_b49[:, 2048:T], a['wav_b49'][:, 2048:T])

            def load(name, shape, dtyp=bf16):
                t = cp.tile(shape, dtyp, tag=name)
                nc.sync.dma_start(t[:], a[name][:])
                return t

            G2 = load('G2init', [17, 49])
            lhsK = load('lhsK', [49, 48])
            E_sl = load('E_slice', [17, QN])
            lhsQ = load('lhsQ', [17, 48])
            ones16 = load('ones16', [16, 1])
            ones49 = cp.tile([48, 1], bf16, tag="ones49")
            nc.sync.dma_start(ones49[0:16, :], a['ones16'][:])
            nc.sync.dma_start(ones49[32:48, :], a['ones16'][:])
            rhsV49 = load('rhsV49', [49, 66])
            W3A = load('W3A', [17, 16])
            W3B = load('W3B', [17, 16])

            # 1. wavPT = ET_dup * wav_exp
            wavPT = cp.tile([128, 1024], bf16, tag="wavPT")
            nc.vector.tensor_tensor(wavPT[:, 0:512], ET_dup[:, 0:512],
                                    wav_exp[:, 0:512], op=ALU.mult)
            nc.vector.tensor_tensor(wavPT[:, 512:1024], ET_dup[:, 512:1024],
                                    wav_exp[:, 512:1024], op=ALU.mult)

            # 2. G_AB [16, 32] -> G2 rows 0:16 (cols 0:16 A, 32:48 B)
            gps = psS.tile([16, 32], f32, tag="S")
            for g in range(32):
                nc.tensor.matmul(gps[:], ET_dup[:, 32 * g:32 * g + 16],
                                 wavPT[:, 32 * g:32 * g + 32],
                                 start=(g == 0), stop=(g == 31))
            nc.vector.tensor_copy(G2[0:16, 0:16], gps[:, 0:16])
            nc.vector.tensor_copy(G2[0:16, 32:48], gps[:, 16:32])

            # 3. wavP2 [49, T] bf16 = (G2.T @ E_aug) * wav_b49 (ones rows via G2 row 16)
            wavP2 = cp.tile([49, T], bf16, tag="wavP2")
            for j in range(8):
                geps = psP.tile([49, 512], f32, tag="P")
                nc.tensor.matmul(geps[:], G2[:], E_aug[:, 512 * j:512 * (j + 1)],
                                 start=True, stop=True)
                nc.vector.tensor_tensor(wavP2[:, 512 * j:512 * (j + 1)], geps[:],
                                        wav_b49[:, 512 * j:512 * (j + 1)], op=ALU.mult)

            # 4. KTall [49, T] bf16: rows 0-15 KT_A, 16 neg6, 32-47 KT_B, 48 neg6
            KTall = cp.tile([49, T], bf16, tag="KTall")
            for j in range(8):
                kps = psP.tile([48, 512], f32, tag="P")
                nc.tensor.matmul(kps[:], lhsK[:], wavP2[:, 512 * j:512 * (j + 1)],
                                 start=True, stop=True)
                nc.vector.tensor_scalar(KTall[0:48, 512 * j:512 * (j + 1)], kps[:],
                                        0.0, 6.0, ALU.max, ALU.min)
            nc.sync.dma_start(KTall[16:17, :], a['neg6'][:])
            nc.sync.dma_start(KTall[48:49, :], a['neg6'][:])

            # 5. QTall [49, QN] bf16: rows 0-15 Q_A, 16 sumQ_A, 32-47 Q_B, 48 sumQ_B
            QTall = cp.tile([49, QN], bf16, tag="QTall")
            qp1 = psP.tile([48, 512], f32, tag="P")
            qp2 = psS.tile([48, 2], f32, tag="S")
            nc.tensor.matmul(qp1[:], lhsQ[:], E_sl[:, 0:512], start=True, stop=True)
            nc.tensor.matmul(qp2[:], lhsQ[:], E_sl[:, 512:QN], start=True, stop=True)
            nc.vector.tensor_scalar(QTall[0:48, 0:512], qp1[:], 0.0, 6.0,
                                    ALU.max, ALU.min)
            nc.vector.tensor_scalar(QTall[0:48, 512:QN], qp2[:], 0.0, 6.0,
                                    ALU.max, ALU.min)
            for bi, lo in ((0, 0), (1, 32)):
                sq1 = psP.tile([1, 512], f32, tag="P")
                sq2 = psS.tile([1, 2], f32, tag="S")
                nc.tensor.matmul(sq1[:], ones49[lo:lo + 16, :], QTall[lo:lo + 16, 0:512],
                                 start=True, stop=True)
                nc.tensor.matmul(sq2[:], ones49[lo:lo + 16, :], QTall[lo:lo + 16, 512:QN],
                                 start=True, stop=True)
                sqb = wp.tile([1, QN], bf16, tag="sqb")
                nc.vector.tensor_copy(sqb[:, 0:512], sq1[:])
                nc.vector.tensor_copy(sqb[:, 512:QN], sq2[:])
                nc.sync.dma_start(QTall[lo + 16:lo + 17, :], sqb[:])

            # V for both blocks
            Vt = cp.tile([128, 32 * 66], bf16, tag="Vt")
            for g in range(32):
                vps = psP.tile([128, 66], f32, tag="P")
                nc.tensor.matmul(vps[:], wavP2[:, 128 * g:128 * (g + 1)],
                                 rhsV49[:], start=True, stop=True)
                nc.vector.tensor_scalar(Vt[:, 66 * g:66 * g + 66], vps[:],
                                        0.0, 6.0, ALU.max, ALU.min)

            # y48 assembly target
            y48 = cp.tile([49, QN], bf16, tag="y48")
            nc.sync.dma_start(y48[0:16, :], a['E_slice'][0:16, :])
            nc.sync.dma_start(y48[48:49, :], a['E_slice'][16:17, :])

            # interleaved paired scores loop.  halo psum [128, 512] regions:
            #   0:64 scoresH-A, 64:128 scoresH-B, 128:130 Uh-A, 130:132 Uh-B
            halo = psH.tile([128, 512], f32, tag="H")
            UA = psUA.tile([33, 512], f32, tag="UA")
            UB = psUB.tile([33, 512], f32, tag="UB")
            for p in range(16):
                for bi, (lo, U) in enumerate(((0, UA), (32, UB))):
                    g0, g1 = 2 * p, 2 * p + 1
                    pair = psP.tile([128, 1024], f32, tag="P")
                    nc.tensor.matmul(pair[:, 0:512], KTall[lo:lo + 17, 128 * g0:128 * g0 + 128],
                                     QTall[lo:lo + 17, 0:512], start=True, stop=True)
                    nc.tensor.matmul(pair[:, 512:1024], KTall[lo:lo + 17, 128 * g1:128 * g1 + 128],
                                     QTall[lo:lo + 17, 0:512], start=True, stop=True)
                    nc.tensor.matmul(halo[:, 64 * bi + 2 * g0:64 * bi + 2 * g0 + 2],
                                     KTall[lo:lo + 17, 128 * g0:128 * g0 + 128],
                                     QTall[lo:lo + 17, 512:QN], start=True, stop=True)
                    nc.tensor.matmul(halo[:, 64 * bi + 2 * g1:64 * bi + 2 * g1 + 2],
                                     KTall[lo:lo + 17, 128 * g1:128 * g1 + 128],
                                     QTall[lo:lo + 17, 512:QN], start=True, stop=True)
                    ex = ep.tile([128, 1024], bf16, tag="ex")
                    nc.scalar.activation(ex[:], pair[:], AF.Exp)
                    nc.tensor.matmul(U[:, 0:512], Vt[:, 66 * g0 + 33 * bi:66 * g0 + 33 * bi + 33],
                                     ex[:, 0:512], start=(g0 == 0), stop=False)
                    nc.tensor.matmul(U[:, 0:512], Vt[:, 66 * g1 + 33 * bi:66 * g1 + 33 * bi + 33],
                                     ex[:, 512:1024], start=False, stop=(g1 == 31))
            # halo exp + halo AV accumulation per block
            for bi in (0, 1):
                exh = ep.tile([128, 64], bf16, tag="exh")
                nc.scalar.activation(exh[:], halo[:, 64 * bi:64 * bi + 64], AF.Exp)
                for g in range(32):
                    nc.tensor.matmul(halo[0:33, 128 + 2 * bi:130 + 2 * bi],
                                     Vt[:, 66 * g + 33 * bi:66 * g + 33 * bi + 33],
                                     exh[:, 2 * g:2 * g + 2],
                                     start=(g == 0), stop=(g == 31))

            # Z stage per block
            for bi, (U, W3) in enumerate(((UA, W3A), (UB, W3B))):
                uh = halo[:, 128 + 2 * bi:130 + 2 * bi]
                rU = wp.tile([1, QN], f32, tag="rU")
                nc.vector.reciprocal(rU[:, 0:512], U[32:33, :])
                nc.vector.reciprocal(rU[:, 512:QN], uh[32:33, :])
                rUb = wp.tile([16, QN], f32, tag="rUb")
                nc.gpsimd.partition_broadcast(rUb[:], rU[:])
                AVn = wp.tile([16, QN], f32, tag="AVn")
                nc.vector.tensor_tensor(AVn[:, 0:512], U[0:16, :], rUb[:, 0:512], op=ALU.mult)
                nc.vector.tensor_tensor(AVn[:, 512:QN], uh[0:16, :], rUb[:, 512:QN], op=ALU.mult)
                Z = wp.tile([17, QN], bf16, tag="Z")
                nc.scalar.activation(Z[0:16, :], AVn[:], AF.Exp)
                dn1 = psP.tile([1, 512], f32, tag="P")
                dn2 = psS.tile([1, 2], f32, tag="S")
                nc.tensor.matmul(dn1[:], ones16[:], Z[0:16, 0:512], start=True, stop=True)
                nc.tensor.matmul(dn2[:], ones16[:], Z[0:16, 512:QN], start=True, stop=True)
                rd = wp.tile([1, QN], f32, tag="rd")
                nc.vector.reciprocal(rd[:, 0:512], dn1[:])
                nc.vector.reciprocal(rd[:, 512:QN], dn2[:])
                dnb = wp.tile([1, QN], bf16, tag="dnb")
                nc.vector.tensor_copy(dnb[:, 0:512], dn1[:])
                nc.vector.tensor_copy(dnb[:, 512:QN], dn2[:])
                nc.sync.dma_start(Z[16:17, :], dnb[:])
                o31 = psP.tile([16, 512], f32, tag="P")
                o32 = psS.tile([16, 2], f32, tag="S")
                nc.tensor.matmul(o31[:], W3[:], Z[:, 0:512], start=True, stop=True)
                nc.tensor.matmul(o32[:], W3[:], Z[:, 512:QN], start=True, stop=True)
                rdb = wp.tile([16, QN], f32, tag="rdb")
                nc.gpsimd.partition_broadcast(rdb[:], rd[:])
                wavm = wp.tile([16, QN], f32, tag="wavm")
                nc.vector.tensor_tensor(wavm[:, 0:512], o31[:], rdb[:, 0:512], op=ALU.mult)
                nc.vector.tensor_tensor(wavm[:, 512:QN], o32[:], rdb[:, 512:QN], op=ALU.mult)
                wavc = wp.tile([16, QN], bf16, tag="wavc")
                nc.vector.tensor_scalar(wavc[:], wavm[:], 0.0, 6.0, ALU.max, ALU.min)
                nc.sync.dma_start(y48[16 + 16 * bi:32 + 16 * bi, :], wavc[:])

            # conv weights load late
            c0w = []
            for dw in range(3):
                t = cp.tile([49, 120], bf16, tag=f"c0w{dw}")
                nc.sync.dma_start(t[:], a['c0w'][dw])
                c0w.append(t)
            c1w = []
            for dw in range(4):
                t = cp.tile([121, 60], bf16, tag=f"c1w{dw}")
                nc.sync.dma_start(t[:], a['c1w'][dw])
                c1w.append(t)
            c2w = []
            for dw in range(4):
                t = cp.tile([61, 30], bf16, tag=f"c2w{dw}")
                nc.sync.dma_start(t[:], a['c2w'][dw])
                c2w.append(t)

            # conv0
            y0 = cp.tile([121, 516], bf16, tag="y0")
            c0ps = psP.tile([120, 512], f32, tag="P")
            for dw in range(3):
                nc.tensor.matmul(c0ps[:], c0w[dw][:], y48[:, dw:dw + 512],
                                 start=(dw == 0), stop=(dw == 2))
            nc.vector.tensor_scalar(y0[0:120, 0:512], c0ps[:], 0.0, 6.0, ALU.max, ALU.min)
            nc.sync.dma_start(y0[120:121, 0:512], a['E_slice'][16:17, 0:512])
            # conv1
            y1 = cp.tile([61, 132], bf16, tag="y1")
            c1ps = psP.tile([60, 128], f32, tag="P")
            for dw in range(4):
                rhs = y0[:, dw:dw + 4 * 128].rearrange("p (n s) -> p n s", s=4)[:, :, 0]
                nc.tensor.matmul(c1ps[:], c1w[dw][:], rhs, start=(dw == 0), stop=(dw == 3))
            nc.vector.tensor_scalar(y1[0:60, 0:128], c1ps[:], 0.0, 6.0, ALU.max, ALU.min)
            nc.sync.dma_start(y1[60:61, 0:128], a['E_slice'][16:17, 0:128])
            # conv2
            y2 = wp.tile([30, 32], f32, tag="y2")
            c2ps = psP.tile([30, 32], f32, tag="P")
            for dw in range(4):
                rhs = y1[:, dw:dw + 4 * 32].rearrange("p (n s) -> p n s", s=4)[:, :, 0]
                nc.tensor.matmul(c2ps[:], c2w[dw][:], rhs, start=(dw == 0), stop=(dw == 3))
            nc.vector.tensor_scalar(y2[:], c2ps[:], 0.0, 6.0, ALU.max, ALU.min)
            nc.sync.dma_start(a['oconv2'][:], y2[:])
    nc.compile()
    return nc


# ---------------------------------------------------------------- launch 2
def _build_launch2():
    nc = bacc.Bacc("TRN2", target_bir_lowering=False, debug=False, num_devices=1)
    dt = nc.dram_tensor
    y2a_ap = dt('y2a', [31, 255], f32, kind="ExternalInput").ap()
    c3w_ap = dt('c3w', [4, 31, 15], f32, kind="ExternalInput").ap()
    f1w_ap = dt('f1w', [31, 15], f32, kind="ExternalInput").ap()
    f2w_ap = dt('f2w', [16, 2], f32, kind="ExternalInput").ap()
    out_ap = dt('out', [42, 2], f32, kind="ExternalOutput").ap()
    scr_ap = dt('scratch', [15, 84], f32).ap()

    with tile.TileContext(nc) as tc:
        with tc.tile_pool(name="sb", bufs=1) as sp, \
             tc.tile_pool(name="ps", bufs=2, space="PSUM") as pp:
            y2a = sp.tile([31, 255], f32)
            f1w = sp.tile([31, 15], f32)
            f2w = sp.tile([16, 2], f32)
            nc.sync.dma_start(y2a[:], y2a_ap[:])
            nc.sync.dma_start(f1w[:], f1w_ap[:])
            nc.sync.dma_start(f2w[:], f2w_ap[:])
            c3w = []
            for dw in range(4):
                t = sp.tile([31, 15], f32, tag=f"c3w{dw}")
                nc.sync.dma_start(t[:], c3w_ap[dw])
                c3w.append(t)
            c3ps = pp.tile([15, 84], f32)
            for dw in range(4):
                rhs = y2a[:, dw:dw + 3 * 84].rearrange("p (n s) -> p n s", s=3)[:, :, 0]
                nc.tensor.matmul(c3ps[:], c3w[dw][:], rhs, start=(dw == 0), stop=(dw == 3))
            y3 = sp.tile([15, 84], f32)
            nc.vector.tensor_scalar(y3[:], c3ps[:], 0.0, 6.0, ALU.max, ALU.min)
            nc.sync.dma_start(scr_ap[:], y3[:])
            # reload as [30, 42] transposed-flat + ones row
            y42T = sp.tile([31, 42], f32)
            flat = scr_ap.rearrange("a b -> (a b)").rearrange("(r m) -> m r", m=30)
            nc.sync.dma_start(y42T[0:30, :], flat)
            nc.sync.dma_start(y42T[30:31, :], y2a_ap[30:31, 0:42])
            p1 = pp.tile([15, 42], f32)
            nc.tensor.matmul(p1[:], f1w[:], y42T[:], start=True, stop=True)
            e1 = sp.tile([15, 42], f32)
            nc.scalar.activation(e1[:], p1[:], AF.Exp, scale=-1.0)
            h = sp.tile([16, 42], f32)
            nc.vector.tensor_scalar(h[0:15, :], e1[:], 1.0, None, ALU.add)
            nc.vector.reciprocal(h[0:15, :], h[0:15, :])
            nc.sync.dma_start(h[15:16, :], y2a_ap[30:31, 0:42])
            p2 = pp.tile([2, 42], f32)
            nc.tensor.matmul(p2[:], f2w[:], h[:], start=True, stop=True)
            e2 = sp.tile([2, 42], f32)
            nc.scalar.activation(e2[:], p2[:], AF.Exp, scale=-1.0)
            e2p = sp.tile([2, 42], f32)
            nc.vector.tensor_scalar(e2p[:], e2[:], 1.0, None, ALU.add)
            o = sp.tile([2, 42], f32)
            nc.vector.reciprocal(o[:], e2p[:])
            nc.sync.dma_start(out_ap.rearrange("r c -> c r"), o[:])
    nc.compile()
    return nc


_NC1 = None
_NC2 = None


def _ensure_built():
    global _NC1, _NC2
    if _NC1 is None:
        _NC1 = _build_launch1()
    if _NC2 is None:
        _NC2 = _build_launch2()


def _run(inputs, trace=False, trace_cores=None):
    _ensure_built()
    c = build_consts(**inputs)
    bf_keys = ('E_aug', 'ET_dup', 'wav_exp', 'wav_b49', 'lhsK', 'rhsV49',
               'lhsQ', 'c0w', 'c1w', 'c2w', 'G2init')
    shared = {k: c[k].astype(BF) for k in bf_keys}
    for k in ('ones16', 'neg6', 'W3A', 'W3B'):
        shared[k] = c[k]
    in_maps = [{**shared, 'E_slice': c['E_slice'][ci].astype(BF)}
               for ci in range(NC)]
    res1 = run_bass_kernel_spmd(_NC1, in_maps, list(range(NC)), trace=trace,
                                trace_cores=trace_cores)
    y2full = np.concatenate([res1.results[ci]['oconv2'] for ci in range(NC)],
                            axis=1)[:, :255]
    y2a = np.concatenate([y2full, np.ones((1, 255), np.float32)], 0)
    in2 = [{'y2a': y2a, 'c3w': c['c3w'], 'f1w': c['f1w'], 'f2w': c['f2w']}]
    res2 = run_bass_kernel_spmd(_NC2, in2, [0], trace=trace)
    out = np.asarray(res2.results[0]['out'], np.float32)
    return out, res1, res2


def kernel(**inputs) -> np.ndarray:
    out, _, _ = _run(inputs, trace=False)
    return out
